# revision 1
# baseline (speedup 1.0000x reference)
"""Trainium2 Bass kernel for 2-layer GraphSAGE (mean aggregation) on 8 NeuronCores.

Strategy (graph/data parallel, dst-partitioned):
  - Destination nodes sharded across 8 cores (12.5K each); edges partitioned by
    destination core and sorted by (dest-tile, src-quarter) on the host.
  - Source features gathered from DRAM quarter-tables with SWDGE dma_gather
    (int16 indices, 512B rows), 4 SWDGE queues round-robin.
  - Segment-mean computed as a one-hot mask matmul on the TensorEngine with
    float32r (full-rate fp32): aggT[f, d] += g_chunk.T @ mask, where
    mask[e, d] = (iota[d] == dst_local[e]) * invdeg[e] is built by one
    DVE tensor_scalar op per 128-edge chunk.
  - Dense branch h = agg @ Wl.T + x @ Wr.T done feature-major; x^T obtained by
    PE-transposing the core's own shard rows.
  - Between layers: AllGather of the h shard into 4 shared quarter-tables so
    layer 2 can gather remote rows; quarter-wise AG overlaps layer-2 compute.
"""

import numpy as np

# ---------------------------------------------------------------- problem dims
N_NODES = 100000
N_EDGES = 800000
D = 128
NC = 8

TILE_D = 512          # destination-tile width (matmul moving free dim)
STILE_SIZES = (7, 6, 6, 6)  # dest-tiles per gather super-group
GATHER_BUFS = 4
NQ = 4                # SWDGE queues

_cache = {}


# ------------------------------------------------------------------- host plan
def _plan(edge_index, n_nodes, n_cores, tile_d, stile_sizes):
    """Partition + sort edges, compute padded per-group layouts shared by all
    cores, and build per-core index/mask streams."""
    src = np.asarray(edge_index[0], dtype=np.int64)
    dst = np.asarray(edge_index[1], dtype=np.int64)
    E = src.shape[0]

    nloc = n_nodes // n_cores
    quart = nloc // 4
    tbl_rows = quart * n_cores
    nt = -(-nloc // tile_d)               # tiles per core
    nloc_pad = nt * tile_d

    # stile partition of tiles
    stiles = []
    t0 = 0
    for s in stile_sizes:
        stiles.append(list(range(t0, min(t0 + s, nt))))
        t0 += s
    stiles = [s for s in stiles if s]
    assert sum(len(s) for s in stiles) == nt

    deg = np.bincount(dst, minlength=n_nodes).astype(np.float64)
    invdeg = (1.0 / np.maximum(deg, 1.0)).astype(np.float32)

    core = dst // nloc
    dloc = dst % nloc
    tile = dloc // tile_d
    dtl = (dloc % tile_d).astype(np.float32)
    srcm = src % nloc
    chunk = srcm // quart
    tblrow = (src // nloc) * quart + srcm % quart   # row within quarter-table
    assert tblrow.max() < tbl_rows

    # group id = (core, tile, chunk); count group sizes
    gid = (core * nt + tile) * 4 + chunk
    order = np.argsort(gid, kind="stable")
    counts = np.bincount(gid, minlength=n_cores * nt * 4).reshape(n_cores, nt, 4)
    # padded group sizes, shared across cores
    gmax = counts.max(axis=0)                       # [nt, 4]
    gpad = ((gmax + 127) // 128) * 128              # multiple of 128
    ep = int(gpad.sum())                            # padded stream length/core

    # stream position of each group, in (stile, chunk, tile) order
    goff = np.zeros((nt, 4), dtype=np.int64)
    pos = 0
    call_list = []                                  # (chunk, [tiles], off, n)
    for tiles in stiles:
        for c in range(4):
            call_off = pos
            for t in tiles:
                goff[t, c] = pos
                pos += int(gpad[t, c])
            call_list.append((c, tiles, call_off, pos - call_off))
    assert pos == ep

    # per-core streams
    idx_st = np.zeros((n_cores, ep), dtype=np.int16)
    dst_st = np.full((n_cores, ep), -1.0, dtype=np.float32)
    inv_st = np.zeros((n_cores, ep), dtype=np.float32)

    gid_s = gid[order]
    put = np.empty(E, dtype=np.int64)
    # position of each sorted edge inside its group
    grp_start = np.searchsorted(gid_s, np.arange(n_cores * nt * 4))
    within = np.arange(E) - grp_start[gid_s]
    k_s = gid_s // (nt * 4)
    t_s = (gid_s // 4) % nt
    c_s = gid_s % 4
    put = goff[t_s, c_s] + within
    idx_st[k_s, put] = tblrow[order].astype(np.int16)
    dst_st[k_s, put] = dtl[order]
    inv_st[k_s, put] = invdeg[dst[order]]

    return dict(
        nloc=nloc, quart=quart, tbl_rows=tbl_rows, nt=nt, nloc_pad=nloc_pad,
        stiles=stiles, gpad=gpad, goff=goff, ep=ep, call_list=call_list,
        idx_st=idx_st, dst_st=dst_st, inv_st=inv_st, tile_d=tile_d,
        n_cores=n_cores, n_nodes=n_nodes,
    )


def _wrap16(stream):
    """[ep] -> [128, ep//16] wrapped-16 + replicated layout for dma_gather."""
    ep = stream.shape[0]
    w = stream.reshape(ep // 16, 16).T          # [16, ep//16]
    return np.tile(w, (8, 1))                   # [128, ep//16]


def _colmajor(stream):
    """[ep] -> [128, ep//128] with element j at [j%128, j//128]."""
    ep = stream.shape[0]
    return stream.reshape(ep // 128, 128).T.copy()


def _make_tables(x_full, plan):
    """x rows -> 4 quarter-tables with row (n) -> table[(n%nloc)//quart],
    row (n//nloc)*quart + (n%nloc)%quart."""
    nloc, quart, tbl = plan["nloc"], plan["quart"], plan["tbl_rows"]
    n_cores = plan["n_cores"]
    tabs = []
    xr = x_full.reshape(n_cores, nloc, D)
    for q in range(4):
        tabs.append(np.ascontiguousarray(
            xr[:, q * quart:(q + 1) * quart, :].reshape(tbl, D)))
    return tabs


# --------------------------------------------------------------- bass builder
def _build(plan, iters=1):
    import os
    SKIP_AG = bool(int(os.environ.get("K_SKIP_AG", "0")))
    SKIP_MASK = bool(int(os.environ.get("K_SKIP_MASK", "0")))
    SKIP_MM = bool(int(os.environ.get("K_SKIP_MM", "0")))
    SKIP_DENSE = bool(int(os.environ.get("K_SKIP_DENSE", "0")))
    MASK_MODE = int(os.environ.get("K_MASK_MODE", "0"))
    import concourse.bass as bass
    import concourse.tile as tile
    from concourse import bacc, mybir
    from concourse.library_config import mlp
    from concourse.tile_rust import add_dep_helper

    f32 = mybir.dt.float32
    f32r = mybir.dt.float32r
    i16 = mybir.dt.int16

    nloc = plan["nloc"]; quart = plan["quart"]; tbl = plan["tbl_rows"]
    nt = plan["nt"]; nloc_pad = plan["nloc_pad"]; td = plan["tile_d"]
    ep = plan["ep"]; gpad = plan["gpad"]; goff = plan["goff"]
    call_list = plan["call_list"]; stiles = plan["stiles"]
    n_cores = plan["n_cores"]
    nblk = td // 128                     # 128-blocks per dest tile
    max_call = max(n for (_, _, _, n) in call_list)

    nc = bacc.Bacc("TRN2", target_bir_lowering=False, debug=False,
                   num_swdge_queues=NQ)

    # inputs
    x_tabs = [nc.dram_tensor(f"x_tab{q}", [tbl, D], f32, kind="ExternalInput")
              for q in range(4)]
    x_shard = nc.dram_tensor("x_shard", [nloc_pad, D], f32, kind="ExternalInput")
    idxs_in = nc.dram_tensor("idxs", [128, ep // 16], i16, kind="ExternalInput")
    dstv_in = nc.dram_tensor("dstv", [128, ep // 128], f32, kind="ExternalInput")
    invv_in = nc.dram_tensor("invv", [128, ep // 128], f32, kind="ExternalInput")
    iota_in = nc.dram_tensor("iota", [128, td], f32, kind="ExternalInput")
    ident_in = nc.dram_tensor("ident", [128, 128], f32, kind="ExternalInput")
    w_in = {}
    for nm in ("w1lt", "w1rt", "w2lt", "w2rt"):
        w_in[nm] = nc.dram_tensor(nm, [128, 128], f32, kind="ExternalInput")
    b_in = {nm: nc.dram_tensor(nm, [128, 1], f32, kind="ExternalInput")
            for nm in ("b1", "b2")}
    out_t = nc.dram_tensor("outT", [128, nloc_pad], f32, kind="ExternalOutput")

    # internal DRAM
    h_shard = nc.dram_tensor("h_shard", [nloc_pad, D], f32)
    h_tabs = [nc.dram_tensor(f"h_tab{q}", [tbl, D], f32, addr_space="Shared")
              for q in range(4)]

    with tile.TileContext(nc) as tc:
        lib_inst = nc.gpsimd.load_library(mlp)
        with (
            tc.tile_pool(name="persist", bufs=1) as pp,
            tc.tile_pool(name="gather", bufs=GATHER_BUFS) as gpo,
            tc.tile_pool(name="mask", bufs=3) as mpo,
            tc.tile_pool(name="aggT", bufs=max(len(s) for s in stiles) + 1) as apo,
            tc.tile_pool(name="small", bufs=2) as spo,
            tc.tile_pool(name="psA", bufs=2, space="PSUM") as psa,
            tc.tile_pool(name="psB", bufs=2, space="PSUM") as psb,
            tc.tile_pool(name="psC", bufs=2, space="PSUM") as psc,
        ):
            # persistent SBUF
            idx_sb = pp.tile([128, ep // 16], i16)
            nc.sync.dma_start(idx_sb[:], idxs_in[:])
            dstv_sb = pp.tile([128, ep // 128], f32)
            nc.sync.dma_start(dstv_sb[:], dstv_in[:])
            invv_sb = pp.tile([128, ep // 128], f32)
            nc.sync.dma_start(invv_sb[:], invv_in[:])
            iota_sb = pp.tile([128, td], f32)
            nc.sync.dma_start(iota_sb[:], iota_in[:])
            ident_sb = pp.tile([128, 128], f32)
            nc.sync.dma_start(ident_sb[:], ident_in[:])
            w_sb = {}
            for nm, t in w_in.items():
                w_f = pp.tile([128, 128], f32, tag=nm + "f", name=f"wf_{nm}")
                nc.sync.dma_start(w_f[:], t[:])
                w_sb[nm] = pp.tile([128, 128], f32r, tag=nm, name=f"w_{nm}")
                nc.vector.tensor_copy(w_sb[nm][:], w_f[:])
            b_sb = {}
            for nm, t in b_in.items():
                b_sb[nm] = pp.tile([128, 1], f32, tag=nm, name=f"b_{nm}")
                nc.sync.dma_start(b_sb[nm][:], t[:])

            first_gather = [True]
            const_m = pp.tile([128, td], f32r, tag="constm", name="constm")
            nc.vector.tensor_copy(const_m[:], iota_sb[:])

            def layer(src_tabs, self_src, wl, wr, bias, is_last, ag_insts):
                """Emit one SAGE layer. Returns list of h-store instructions."""
                store_insts = []
                for tiles in stiles:
                    # issue the stile's 4 gather calls (parallel queues)
                    gbufs = {}
                    ginsts = {}
                    for (c, ctiles, off, n) in call_list:
                        if ctiles is not tiles:
                            continue
                        g = gpo.tile([128, max_call // 128, D], f32r, tag="g")
                        slots = n // 128
                        gi = nc.gpsimd.dma_gather(
                            g[:, :slots, :], src_tabs[c][:].bitcast(f32r),
                            idx_sb[:, off // 16:(off + n) // 16],
                            n, n, D, queue_num=c,
                            single_packet=False)
                        if first_gather[0]:
                            add_dep_helper(gi.ins, lib_inst.ins, sync=True,
                                           reason="lib before gather")
                            first_gather[0] = False
                        if ag_insts is not None:
                            add_dep_helper(gi.ins, ag_insts[c].ins, sync=True,
                                           reason="gather after AG")
                        gbufs[c] = (g, off)
                        ginsts[c] = gi
                    # consume chunk-major: per chunk, all its tiles' groups
                    aggs = {}
                    for c in range(4):
                        g, off = gbufs[c]
                        for t in tiles:
                            npads = int(gpad[t, c])
                            if npads == 0:
                                continue
                            base = int(goff[t, c]) - off      # within call
                            ps = psa.tile([128, td], f32, tag="psagg")
                            nchunks = npads // 128
                            for ci in range(nchunks):
                                col = (off + base) // 128 + ci
                                if SKIP_MASK:
                                    m = const_m
                                else:
                                    mdt = f32 if MASK_MODE in (1, 3) else f32r
                                    m = mpo.tile([128, td], mdt, tag="m")
                                    s1 = dstv_sb[:, col:col + 1] if MASK_MODE in (0, 1) else 0.5
                                    s2 = invv_sb[:, col:col + 1] if MASK_MODE in (0, 1) else 0.25
                                    nc.vector.tensor_scalar(
                                        m[:], iota_sb[:], s1, s2,
                                        mybir.AluOpType.is_equal,
                                        mybir.AluOpType.mult)
                                if not SKIP_MM:
                                    nc.tensor.matmul(
                                        ps[:],
                                        g[:, base // 128 + ci, :],
                                        m[:],
                                        start=(ci == 0), stop=(ci == nchunks - 1))
                            if SKIP_MM:
                                continue
                            if t not in aggs:
                                aggs[t] = apo.tile([128, td], f32r, tag="agg", name=f"agg_t{t}")
                                nc.vector.tensor_copy(aggs[t][:], ps[:])
                            else:
                                nc.vector.tensor_add(aggs[t][:], aggs[t][:], ps[:])
                    # dense + activation + store per tile
                    for t in tiles:
                        if SKIP_DENSE:
                            continue
                        if t not in aggs:
                            aggs[t] = apo.tile([128, td], f32r, tag="agg", name=f"agg_t{t}")
                            nc.vector.tensor_copy(aggs[t][:], const_m[:])
                        # self rows -> selfT via PE transpose
                        xr = spo.tile([128, nblk, 128], f32, tag="xr")
                        nc.sync.dma_start(
                            xr[:],
                            self_src[t * td:(t + 1) * td, :].rearrange(
                                "(a p) f -> p a f", p=128))
                        selfT = spo.tile([128, td], f32r, tag="selfT")
                        for a in range(nblk):
                            tp = psb.tile([128, 128], f32, tag="pst")
                            nc.tensor.transpose(tp[:], xr[:, a, :], ident_sb[:])
                            nc.scalar.copy(selfT[:, a * 128:(a + 1) * 128], tp[:])
                        hp = psc.tile([128, td], f32, tag="psh")
                        agg_t = aggs.get(t)
                        nc.tensor.matmul(hp[:], wl[:], agg_t[:],
                                         start=True, stop=False)
                        nc.tensor.matmul(hp[:], wr[:], selfT[:],
                                         start=False, stop=True)
                        hT = spo.tile([128, td], f32, tag="hT")
                        if is_last:
                            nc.vector.tensor_scalar_add(hT[:], hp[:], bias[:])
                            si = nc.sync.dma_start(
                                out_t[:, t * td:(t + 1) * td], hT[:])
                            store_insts.append(si)
                        else:
                            sg = spo.tile([128, td], f32, tag="sg")
                            nc.scalar.activation(
                                sg[:], hp[:], mybir.ActivationFunctionType.Sigmoid,
                                bias=bias[:])
                            tmp = spo.tile([128, td], f32, tag="tmpb")
                            nc.vector.tensor_scalar_add(tmp[:], hp[:], bias[:])
                            nc.vector.tensor_mul(hT[:], tmp[:], sg[:])
                            # transpose back to row-major and store to h_shard
                            hr = spo.tile([128, nblk, 128], f32, tag="hr")
                            for a in range(nblk):
                                tp = psb.tile([128, 128], f32, tag="pst")
                                nc.tensor.transpose(
                                    tp[:], hT[:, a * 128:(a + 1) * 128],
                                    ident_sb[:])
                                nc.scalar.copy(hr[:, a, :], tp[:])
                            si = nc.sync.dma_start(
                                h_shard[t * td:(t + 1) * td, :].rearrange(
                                    "(a p) f -> p a f", p=128),
                                hr[:])
                            store_insts.append(si)
                return store_insts

            for _ in range(iters):
                l1_stores = layer(x_tabs, x_shard,
                                  w_sb["w1lt"], w_sb["w1rt"], b_sb["b1"],
                                  False, None)
                ag_insts = []
                if SKIP_AG:
                    ag_insts = None
                for q in range(4) if not SKIP_AG else []:
                    ag = nc.gpsimd.collective_compute(
                        "AllGather", mybir.AluOpType.bypass,
                        replica_groups=[list(range(n_cores))],
                        ins=[h_shard[q * quart:(q + 1) * quart, :]],
                        outs=[h_tabs[q][:]])
                    for si in l1_stores:
                        add_dep_helper(ag.ins, si.ins, sync=True,
                                       reason="AG after h stores")
                    ag_insts.append(ag)
                layer(h_tabs, h_shard,
                      w_sb["w2lt"], w_sb["w2rt"], b_sb["b2"],
                      True, ag_insts)

    nc.compile()
    return nc


# ------------------------------------------------------------------ reference-
# shaped entry point
def _prepare(edge_index):
    plan = _plan(edge_index, N_NODES, NC, TILE_D, STILE_SIZES)
    return plan


def _in_maps(plan, x, w1l, w1r, b1, w2l, w2r, b2):
    x = np.ascontiguousarray(np.asarray(x, dtype=np.float32))
    tabs = _make_tables(x, plan)
    nloc, nloc_pad = plan["nloc"], plan["nloc_pad"]
    n_cores = plan["n_cores"]
    td = plan["tile_d"]
    iota = np.broadcast_to(np.arange(td, dtype=np.float32), (128, td)).copy()
    ident = np.eye(128, dtype=np.float32)
    xr = x.reshape(n_cores, nloc, D)
    maps = []
    for k in range(n_cores):
        xs = np.zeros((nloc_pad, D), np.float32)
        xs[:nloc] = xr[k]
        m = {
            "x_shard": xs,
            "idxs": _wrap16(plan["idx_st"][k]),
            "dstv": _colmajor(plan["dst_st"][k]),
            "invv": _colmajor(plan["inv_st"][k]),
            "iota": iota, "ident": ident,
            "w1lt": np.ascontiguousarray(np.asarray(w1l, np.float32).T),
            "w1rt": np.ascontiguousarray(np.asarray(w1r, np.float32).T),
            "w2lt": np.ascontiguousarray(np.asarray(w2l, np.float32).T),
            "w2rt": np.ascontiguousarray(np.asarray(w2r, np.float32).T),
            "b1": np.asarray(b1, np.float32).reshape(128, 1),
            "b2": np.asarray(b2, np.float32).reshape(128, 1),
        }
        for q in range(4):
            m[f"x_tab{q}"] = tabs[q]
        maps.append(m)
    return maps


def _run(inputs, iters=1):
    """Compile (cached) and run; returns full [N, D] output."""
    from concourse.bass_utils import run_bass_kernel_spmd

    edge_index = np.asarray(inputs["edge_index"])
    key = ("k", iters, edge_index.shape[1])
    if key not in _cache:
        plan = _prepare(edge_index)
        nc = _build(plan, iters=iters)
        _cache[key] = (plan, nc)
    plan, nc = _cache[key]
    maps = _in_maps(plan, inputs["x"], inputs["W1_l"], inputs["W1_r"],
                    inputs["b1"], inputs["W2_l"], inputs["W2_r"], inputs["b2"])
    res = run_bass_kernel_spmd(nc, maps, core_ids=list(range(plan["n_cores"])))
    nloc = plan["nloc"]
    outs = [np.asarray(res.results[k]["outT"]).T[:nloc] for k in range(plan["n_cores"])]
    return np.concatenate(outs, axis=0)


def kernel(**inputs) -> np.ndarray:
    return _run(inputs, iters=1)

